# revision 14
# baseline (speedup 1.0000x reference)
"""EnergyAttention Trainium2 kernel (8-core SPMD, head/q hybrid sharding).

reference math:
    K = einsum('kd,hzd->khz', g, Wk); Q = einsum('qd,hzd->qhz', g, Wq)
    scores = beta * einsum('qhz,khz->hqk', Q, K)        # [H, N, N]
    A = logsumexp(scores, -1); out = (-1/beta) * A.sum()

Sharding (no collectives; final scalar reduction on host):
    core c owns head A = c (all 2048 q rows) and head B = 8 + c//2
    restricted to q rows [1024*(c%2), 1024*(c%2)+1024).  Every core runs an
    identical program; only input values differ (SPMD-safe).

Implementation notes:
  - inputs are cast to bf16 on the host (beta folded into Wq); matmuls are
    bf16 with fp32 PSUM accumulation
  - g -> gT rides the DMA crossbar transpose straight out of DRAM (the xbar
    is the serial startup resource, ~261 GB/s); W and gb transpose on the
    otherwise-idle PE via identity matmuls
  - per 128-row q-tile "unit": 4 matmuls -> PSUM scores [128, 2048]; DVE
    reduce_max(negate) -> ACT exp(bias=-m) with fused accum row-sum; these
    two consumer passes (~2.2us/unit) set the steady-state cadence
  - host finishes: A = m + log(l), fp64 sum, scale by -1/beta
"""

import numpy as np
import ml_dtypes
from contextlib import ExitStack

import concourse.bass as bass
import concourse.mybir as mybir
import concourse.tile as tile
from concourse import bacc
from concourse.bass_utils import run_bass_kernel_spmd
from concourse.masks import make_identity

N, D, H, Y = 2048, 768, 12, 64
NCORES = 8
BETA = 1.0 / 8.0
DT = mybir.dt.float32
DTB = mybir.dt.bfloat16


def _units():
    # A-units first (QT_B is produced last), then interleave A/B so adjacent
    # PE matmuls contract on disjoint row groups (A: partitions 0:64, B: 64:128)
    units = [("A", j) for j in range(4)]
    for j in range(8):
        units.append(("B", j))
        units.append(("A", 4 + j))
    units += [("A", j) for j in range(12, 16)]
    return units


def _build_kernel():
    nc = bacc.Bacc("TRN2", target_bir_lowering=False, debug=False, num_devices=1)
    g_ap = nc.dram_tensor("g", [N, D], DTB, kind="ExternalInput").ap()
    gb_ap = nc.dram_tensor("gb", [N // 2, D], DTB, kind="ExternalInput").ap()
    wq_ap = nc.dram_tensor("wq", [128, D], DTB, kind="ExternalInput").ap()
    wk_ap = nc.dram_tensor("wk", [128, D], DTB, kind="ExternalInput").ap()
    out_ap = nc.dram_tensor("stats", [128, 48], DT, kind="ExternalOutput").ap()

    AF = mybir.ActivationFunctionType
    AX = mybir.AxisListType
    OP = mybir.AluOpType

    with tile.TileContext(nc) as tc, ExitStack() as ctx:
        const_pool = ctx.enter_context(tc.tile_pool(name="const", bufs=1))
        ident = const_pool.tile([128, 128], DTB)
        make_identity(nc, ident[:])

        w_pool = ctx.enter_context(tc.tile_pool(name="w", bufs=1))
        wq_sb = w_pool.tile([128, D], DTB)
        nc.sync.dma_start(wq_sb[:], wq_ap[:])
        wk_sb = w_pool.tile([128, D], DTB)
        nc.sync.dma_start(wk_sb[:], wk_ap[:])
        # wt blocks 0..5 = WqT d-tiles ([128 d, 64 zA | 64 zB]), 6..11 = WkT
        wt_sb = w_pool.tile([128, 12 * 128], DTB)

        proj_pool = ctx.enter_context(tc.tile_pool(name="proj", bufs=1))
        kt_sb = proj_pool.tile([128, N], DTB)       # rows 0:64 KT_A, 64:128 KT_B
        qta_sb = proj_pool.tile([64, N], DTB)       # QT of head A, all q
        qtb_sb = proj_pool.tile([128, N // 2], DTB)  # rows 64:128 = QT of head B
        stat_pool = ctx.enter_context(tc.tile_pool(name="stat", bufs=8))

        with tc.tile_pool(name="gsrc", bufs=1) as gsrc_pool, \
             tc.tile_pool(name="gt", bufs=1) as gt_pool:
            # gb arrives via regular DMA (scalar ring, keeps the xbar free)
            gb_sb = gsrc_pool.tile([128, 8, D], DTB)
            gb_r = gb_ap.rearrange("(i p) d -> p i d", p=128)
            for c in range(2):
                nc.sync.dma_start(
                    gb_sb[:, 4 * c : 4 * (c + 1), :], gb_r[:, 4 * c : 4 * (c + 1), :]
                )

            # gT via xbar transpose straight from DRAM (sync ring):
            # gt[c][p, t, i] = g[512c + i, 128t + p]
            gt = []
            for c in range(4):
                gtc = gt_pool.tile([128, 6, 512], DTB, name=f"gt{c}")
                nc.sync.dma_start_transpose(gtc[:], g_ap[512 * c : 512 * (c + 1), :])
                gt.append(gtc)

            gtb_sb = gt_pool.tile([128, 6, N // 2], DTB)

            with tc.tile_pool(name="tp", bufs=4, space="PSUM") as tp, \
                 tc.tile_pool(name="pp", bufs=2, space="PSUM") as pp:
                # ---- W transposes on PE: 12 [128,128] blocks, 4 per PSUM bank
                for grp in range(3):
                    ps = tp.tile([128, 512], DTB, tag="tps", name="ps_w")
                    for j in range(4):
                        blk = grp * 4 + j
                        src = wq_sb if blk < 6 else wk_sb
                        t = blk % 6
                        nc.tensor.transpose(
                            ps[:, 128 * j : 128 * (j + 1)],
                            src[:, 128 * t : 128 * (t + 1)],
                            ident[:],
                        )
                    nc.vector.tensor_copy(wt_sb[:, 512 * grp : 512 * (grp + 1)], ps[:])

                # ---- gb transposes on PE (PE is otherwise idle at startup)
                for c in range(2):
                    for t in range(6):
                        ps = tp.tile([128, 512], DTB, tag="tps", name="ps_gb")
                        for j in range(4):
                            i = 4 * c + j
                            nc.tensor.transpose(
                                ps[:, 128 * j : 128 * (j + 1)],
                                gb_sb[:, i, 128 * t : 128 * (t + 1)],
                                ident[:],
                            )
                        nc.vector.tensor_copy(
                            gtb_sb[:, t, 512 * c : 512 * (c + 1)], ps[:]
                        )

                # ---- projections: KT chunk + QTA chunk as soon as gt[c] lands
                for c in range(4):
                    ps = pp.tile([128, 512], DT, tag="ppk", name="ps_kt")
                    for t in range(6):
                        nc.tensor.matmul(
                            ps[:],
                            lhsT=wt_sb[:, 128 * (6 + t) : 128 * (7 + t)],
                            rhs=gt[c][:, t, :],
                            start=(t == 0),
                            stop=(t == 5),
                        )
                    nc.scalar.copy(kt_sb[:, 512 * c : 512 * (c + 1)], ps[:])
                    ps = pp.tile([128, 512], DT, tag="ppk", name="ps_qta")[0:64, :]
                    for t in range(6):
                        nc.tensor.matmul(
                            ps[:],
                            lhsT=wt_sb[:, 128 * t : 128 * t + 64],
                            rhs=gt[c][:, t, :],
                            start=(t == 0),
                            stop=(t == 5),
                        )
                    nc.scalar.copy(qta_sb[:, 512 * c : 512 * (c + 1)], ps[:])
                # QT of head B at partitions 0:64, then DMA-shift to 64:128
                qtb_lo = proj_pool.tile([64, N // 2], DTB)
                for c in range(2):
                    ps = pp.tile([128, 512], DT, tag="ppk", name="ps_qtb")[0:64, :]
                    for t in range(6):
                        nc.tensor.matmul(
                            ps[:],
                            lhsT=wt_sb[:, 128 * t + 64 : 128 * (t + 1)],
                            rhs=gtb_sb[:, t, 512 * c : 512 * (c + 1)],
                            start=(t == 0),
                            stop=(t == 5),
                        )
                    nc.scalar.copy(qtb_lo[:, 512 * c : 512 * (c + 1)], ps[:])
                nc.sync.dma_start(qtb_sb[64:128, :], qtb_lo[:])

        # ---- scores + logsumexp stats per 128-row q-tile
        with tc.tile_pool(name="sp", bufs=2, space="PSUM") as sp:
            for u, (kind, j) in enumerate(_units()):
                ps = sp.tile([128, N], DT, tag="sps", name="ps_s")
                for c in range(4):
                    if kind == "A":
                        lhsT = qta_sb[:, 128 * j : 128 * (j + 1)]
                        rhs = kt_sb[0:64, 512 * c : 512 * (c + 1)]
                    else:
                        lhsT = qtb_sb[64:128, 128 * j : 128 * (j + 1)]
                        rhs = kt_sb[64:128, 512 * c : 512 * (c + 1)]
                    nc.tensor.matmul(
                        ps[:, 512 * c : 512 * (c + 1)],
                        lhsT=lhsT,
                        rhs=rhs,
                        start=True,
                        stop=True,
                    )
                st = stat_pool.tile([128, 2], DT, tag="st", name="st")
                nc.vector.tensor_reduce(
                    st[:, 0:1], ps[:], axis=AX.X, op=OP.max, negate=True
                )
                nc.scalar.activation(
                    ps[:], ps[:], AF.Exp, bias=st[:, 0:1], scale=1.0,
                    accum_out=st[:, 1:2],
                )
                nc.sync.dma_start(out_ap[:, 2 * u : 2 * (u + 1)], st[:])

    nc.compile()
    return nc


_NC_CACHE = {}


def _get_nc():
    if "nc" not in _NC_CACHE:
        _NC_CACHE["nc"] = _build_kernel()
    return _NC_CACHE["nc"]


def _unit_order():
    return _units()


def _make_in_maps(np_inputs):
    bf16 = ml_dtypes.bfloat16
    g = np.ascontiguousarray(np.asarray(np_inputs["g"], dtype=np.float32).astype(bf16))
    Wq = np.asarray(np_inputs["Wq"], dtype=np.float32) * np.float32(BETA)
    Wk = np.asarray(np_inputs["Wk"], dtype=np.float32)
    in_maps = []
    for c in range(NCORES):
        hb = 8 + c // 2
        qlo = (N // 2) * (c % 2)
        in_maps.append(
            {
                "g": g,
                "gb": np.ascontiguousarray(g[qlo : qlo + N // 2]),
                "wq": np.ascontiguousarray(
                    np.concatenate([Wq[c], Wq[hb]], axis=0).astype(bf16)
                ),
                "wk": np.ascontiguousarray(
                    np.concatenate([Wk[c], Wk[hb]], axis=0).astype(bf16)
                ),
            }
        )
    return in_maps


def kernel(g, Wq, Wk):
    in_maps = _make_in_maps({"g": g, "Wq": Wq, "Wk": Wk})
    nc = _get_nc()
    res = run_bass_kernel_spmd(nc, in_maps, core_ids=list(range(NCORES)))

    total = 0.0
    for c in range(NCORES):
        stats = res.results[c]["stats"].astype(np.float64)  # [128, 48]
        neg_m = stats[:, 0::2]  # [128, 24]
        l = stats[:, 1::2]
        total += (-neg_m + np.log(l)).sum()
    return np.float32(-(1.0 / BETA) * total)
